# revision 5
# baseline (speedup 1.0000x reference)
"""Trainium2 Bass kernel: causal depthwise Conv1d (K=4) + SiLU.

Reference computation (B=4, S=4096, D=2048):
    y[b, s, d] = silu( sum_k w[d, 0, k] * x[b, s-3+k, d] )   (zero-padded left)

Strategy (v2):
  * Host: transpose x to channel-major (D, B, S), left-pad each row with
    4 zeros (row length 4100), cast to bf16, shard D across the 8
    NeuronCores (256 channels each).  Depthwise conv is channel-independent
    -> no inter-core communication.
  * Core: 8 tiles of [128, 4100].  Measured engine rates (NTFF):
      PE:  one 512-col matmul per ~218ns (LDW hidden)  -> 1.71 ns/col/tile
      DVE: 4 tensor_scalar @0.27ns/col + 3 adds @0.53ns/col -> 2.67 ns/col
      ACT: silu 0.92 ns/col + ~170ns/inst
      HBM: ~412 GB/s aggregate; 17.05 MB in+out -> 41.4us floor
    Tiles 0,2,3,5,7 on PE (diag-stationary matmuls, 4-tap accumulation in
    PSUM, ACT silu drains PSUM->bf16), tiles 1,4,6 on DVE.  Units are
    emitted in modeled completion order so the strict-FIFO ACT queue never
    head-of-line blocks.
  * Outputs: per-2048-chunk DMAs; early ones on gpsimd (SWDGE), late ones
    alternate sync/scalar (HWDGE, short completion receipt) so the kernel
    tail closes fast.
  * Host: gather, transpose back, cast to f32.
"""

import os
import sys

sys.path.insert(0, "/opt/trn_rl_repo")

import numpy as np
import ml_dtypes

N_CORES = 8
B, S, D = 4, 4096, 2048
K = 4
PAD = 4
ROW = S + PAD  # 4100
D_LOCAL = D // N_CORES  # 256
G = D_LOCAL // 128  # 2 partition groups per core

MM_N = 512  # PSUM bank limit (512 f32)
WARMUP_MMS = int(os.environ.get("KERNEL_WARMUP", "6"))
PE_CHUNK = int(os.environ.get("KERNEL_PE_CHUNK", "2048"))
# columns of the split tile computed on DVE (rest on PE); 0 = no split
SPLIT_TILE = int(os.environ.get("KERNEL_SPLIT_TILE", "6"))
SPLIT_DVE_COLS = int(os.environ.get("KERNEL_SPLIT_DVE", "4096"))
# modeled-completion threshold (ns) after which outputs use HWDGE queues
HW_OUT_NS = float(os.environ.get("KERNEL_HW_OUT_NS", "34000"))

_CACHE = {}

# ---- measured cost model (ns) for emission ordering ----------------------
PE_START = 8300.0
DVE_START = 9500.0
PE_NS_PER_COL = 1.71  # 4 taps, 218ns per 512-col matmul
DVE_TS_NS = lambda w: w * 0.27 + 170.0
DVE_TT_NS = lambda w: w * 0.53 + 100.0
ACT_NS = lambda w: w * 0.92 + 170.0


def _dve_chain_ns(w):
    return 4 * DVE_TS_NS(w) + 3 * DVE_TT_NS(w)


def _build():
    import concourse.tile as tile
    from concourse import bacc, mybir

    nc = bacc.Bacc("TRN2", debug=False, enable_asserts=False, num_devices=N_CORES)
    bf16 = mybir.dt.bfloat16
    f32 = mybir.dt.float32

    x_ap = nc.dram_tensor("x", [G, 128, B, ROW], bf16, kind="ExternalInput").ap()
    wd_ap = nc.dram_tensor("wd", [128, G * K * 128], bf16, kind="ExternalInput").ap()
    w_ap = nc.dram_tensor("w", [128, G * K], f32, kind="ExternalInput").ap()
    out_ap = nc.dram_tensor("out", [G, 128, B, S], bf16, kind="ExternalOutput").ap()

    DVE_TILES = (1, 4, 6)
    NT = G * B  # 8

    with tile.TileContext(nc) as tc:
        with (
            tc.tile_pool(name="wp", bufs=1) as wp,
            tc.tile_pool(name="xp", bufs=8) as xp,
            tc.tile_pool(name="tp", bufs=2) as tp,
            tc.tile_pool(name="cp", bufs=2) as cp,
            tc.tile_pool(name="ps", bufs=2, space="PSUM") as ps,
            tc.tile_pool(name="yp", bufs=4) as yp,
        ):
            wd = wp.tile([128, G * K * 128], bf16, tag="wd")
            wt = wp.tile([128, G * K], f32, tag="wt")

            def wdiag(g, k):
                c0 = (g * K + k) * 128
                return wd[:, c0 : c0 + 128]

            def wcol(g, k):
                return wt[:, g * K + k : g * K + k + 1]

            # HAM warmup: dummy matmuls on a zeroed stationary keep the PE
            # p-state ramping through the ~3us window so real chunks run at
            # full clock.  Gated only on a gpsimd memset; result never read.
            if WARMUP_MMS:
                zt = wp.tile([128, MM_N], bf16, tag="zt")
                nc.gpsimd.memset(zt[:], 0)
                warm = ps.tile([128, PE_CHUNK], f32, tag="acc")
                for _ in range(WARMUP_MMS):
                    nc.tensor.matmul(
                        warm[:, 0:MM_N], zt[:, 0:128], zt[:], start=True, stop=True
                    )

            # ---- input DMAs (sync queue, HWDGE) --------------------------
            # Tile 0 (PE) streams in 3 chunks for a fast PE ramp; tile 1
            # (DVE) in 2 chunks; the rest are whole-tile 1MB transfers.
            # Chunk bounds sit 4 cols past each compute boundary (a chunk
            # reads up to lo+W+3+1).
            tile_bounds = {}
            for ti in range(NT):
                if ti == 0:
                    tile_bounds[ti] = [0, 1028, 2052, ROW]
                else:
                    tile_bounds[ti] = [0, 2052, ROW]
            xts = [None] * NT
            nc.sync.dma_start(out=wd[:], in_=wd_ap[:])
            nc.sync.dma_start(out=wt[:], in_=w_ap[:])
            in_done_ns = {}  # ti -> list of (col_hi, modeled arrival ns)
            t_in = 7900.0
            IN_RATE = 0.35  # ~350 GB/s early input share (bytes/ns)
            for ti in range(NT):
                g, b = divmod(ti, B)
                xt = xp.tile([128, ROW], bf16, tag="xt")
                xts[ti] = xt
                in_done_ns[ti] = []
                for ci in range(len(tile_bounds[ti]) - 1):
                    c0, c1 = tile_bounds[ti][ci], tile_bounds[ti][ci + 1]
                    nc.sync.dma_start(out=xt[:, c0:c1], in_=x_ap[g, :, b, c0:c1])
                    t_in += (c1 - c0) * 128 * 2 / (IN_RATE * 1000.0)
                    in_done_ns[ti].append((c1, t_in))

            def arrival(ti, col_hi):
                # modeled time the input covering [0, col_hi+PAD) has landed
                for c1, t in in_done_ns[ti]:
                    if c1 >= min(col_hi + PAD, ROW):
                        return t
                return in_done_ns[ti][-1][1]

            # ---- build unit worklist with modeled completion times -------
            # kinds: "pe" (chunk: matmuls), "dvec" (vector chain),
            # then per-2048 "silu" units (ACT) and output DMAs.
            units = []  # (ready_ns, kind, ti, lo, hi)
            t_pe = PE_START
            for ti in range(NT):
                on_dve = ti in DVE_TILES
                if on_dve and not (ti == SPLIT_TILE and SPLIT_DVE_COLS < S):
                    continue
                if ti == 0:
                    chunks = [(0, 1024), (1024, 2048), (2048, 3072), (3072, S)]
                else:
                    lo0 = SPLIT_DVE_COLS if (on_dve and ti == SPLIT_TILE) else 0
                    chunks = []
                    c0 = lo0
                    while c0 < S:
                        chunks.append((c0, min(c0 + PE_CHUNK, S)))
                        c0 += PE_CHUNK
                for lo, hi in chunks:
                    t_pe = max(t_pe, arrival(ti, hi)) + (hi - lo) * PE_NS_PER_COL
                    units.append((t_pe, "pe", ti, lo, hi))

            t_dve = DVE_START
            for ti in DVE_TILES:
                hi_t = SPLIT_DVE_COLS if ti == SPLIT_TILE else S
                if ti == 1:
                    chains = [(0, 2048), (2048, S)]
                else:
                    chains = [(0, hi_t)]
                for lo, hi in chains:
                    t_dve = max(t_dve, arrival(ti, hi)) + _dve_chain_ns(hi - lo)
                    units.append((t_dve, "dvec", ti, lo, hi))

            units.sort(key=lambda u: u[0])

            # ---- expand into per-engine emission streams -----------------
            # ACT (silu) units are emitted in modeled completion order of
            # their producer; each silu is followed by its output DMA.
            work = []  # (order_ns, kind, ti, lo, hi, extra)
            for t_done, kind, ti, lo, hi in units:
                work.append((t_done, kind, ti, lo, hi))
            # silu+out units: 2048 granularity, derived from compute units
            sil = []
            for t_done, kind, ti, lo, hi in units:
                for c0 in range(lo, hi, 2048):
                    c1 = min(c0 + 2048, hi)
                    sil.append((t_done + (c0 - lo) * 0.01 + 1.0, kind, ti, c0, c1))
            sil.sort(key=lambda u: u[0])
            for t_done, kind, ti, lo, hi in sil:
                work.append((t_done + 0.5, "silu", ti, lo, hi))
            work.sort(key=lambda u: u[0])

            cbufs = {}  # (ti, col) -> (c tile, chain lo) for DVE results
            accs = {}  # (ti, lo) -> psum tile for PE results
            hw_q = [nc.sync, nc.scalar]
            hw_i = 0
            last_silu = max(t for t, k, *_ in work if k == "silu")

            def emit_pe(ti, lo, hi):
                g, b = divmod(ti, B)
                xt = xts[ti]
                cw = hi - lo
                acc = ps.tile([128, cw], f32, tag="acc")
                accs[(ti, lo)] = acc
                for k in range(K):
                    for n0 in range(0, cw, MM_N):
                        xlo = lo + n0 + 1 + k
                        nw = min(MM_N, cw - n0)
                        nc.tensor.matmul(
                            acc[:, n0 : n0 + nw],
                            wdiag(g, k),
                            xt[:, xlo : xlo + nw],
                            start=(k == 0),
                            stop=(k == K - 1),
                        )

            def emit_dve_chain(ti, lo, hi):
                g, b = divmod(ti, B)
                xt = xts[ti]
                W = hi - lo
                t0 = tp.tile([128, W], bf16, tag="t0")
                nc.vector.tensor_scalar_mul(t0[:], xt[:, lo + 1 : lo + 1 + W], wcol(g, 0))
                t1 = tp.tile([128, W], bf16, tag="t1")
                nc.vector.tensor_scalar_mul(t1[:], xt[:, lo + 2 : lo + 2 + W], wcol(g, 1))
                p0 = cp.tile([128, W], bf16, tag="p0")
                nc.vector.tensor_add(p0[:], t0[:], t1[:])
                t2 = tp.tile([128, W], bf16, tag="t0")
                nc.vector.tensor_scalar_mul(t2[:], xt[:, lo + 3 : lo + 3 + W], wcol(g, 2))
                t3 = tp.tile([128, W], bf16, tag="t1")
                nc.vector.tensor_scalar_mul(t3[:], xt[:, lo + 4 : lo + 4 + W], wcol(g, 3))
                p1 = cp.tile([128, W], bf16, tag="p1")
                nc.vector.tensor_add(p1[:], t2[:], t3[:])
                c = cp.tile([128, W], bf16, tag="c")
                nc.vector.tensor_add(c[:], p0[:], p1[:])
                for c0 in range(lo, hi, 2048):
                    cbufs[(ti, c0)] = (c, lo)

            def emit_silu_out(t_done, kind, ti, lo, hi):
                nonlocal hw_i
                g, b = divmod(ti, B)
                W = hi - lo
                is_last = t_done >= last_silu - 1e-9
                y = yp.tile([128, W], bf16, tag="y")
                sw = 1024 if is_last else W
                for s0 in range(0, W, sw):
                    scw = min(sw, W - s0)
                    if kind == "pe":
                        src = accs[(ti, lo)]
                        nc.scalar.activation(
                            out=y[:, s0 : s0 + scw],
                            in_=src[:, s0 : s0 + scw],
                            func=mybir.ActivationFunctionType.Silu,
                        )
                    else:
                        c, chain_lo = cbufs[(ti, lo)]
                        o = lo - chain_lo + s0
                        nc.scalar.activation(
                            out=y[:, s0 : s0 + scw],
                            in_=c[:, o : o + scw],
                            func=mybir.ActivationFunctionType.Silu,
                        )
                    if is_last:
                        nc.scalar.dma_start(
                            out=out_ap[g, :, b, lo + s0 : lo + s0 + scw],
                            in_=y[:, s0 : s0 + scw],
                        )
                if not is_last:
                    if t_done < HW_OUT_NS:
                        nc.gpsimd.dma_start(out=out_ap[g, :, b, lo:hi], in_=y[:])
                    else:
                        hw_q[hw_i % 2].dma_start(out=out_ap[g, :, b, lo:hi], in_=y[:])
                        hw_i += 1

            for t_done, kind, ti, lo, hi in work:
                if kind == "pe":
                    emit_pe(ti, lo, hi)
                elif kind == "dvec":
                    emit_dve_chain(ti, lo, hi)
                else:
                    # find producer kind for this silu chunk
                    pk = "pe" if (ti, lo) in accs else "dve"
                    emit_silu_out(t_done, pk, ti, lo, hi)

    nc.compile()
    return nc


def _get_nc():
    if "nc" not in _CACHE:
        _CACHE["nc"] = _build()
    return _CACHE["nc"]


def _make_in_maps(x, w):
    x = np.asarray(x, dtype=np.float32)
    w = np.asarray(w, dtype=np.float32)

    # (B, S, D) -> (D, B, S), bf16, left-pad rows with PAD zeros.
    x_t = np.ascontiguousarray(x.transpose(2, 0, 1)).astype(ml_dtypes.bfloat16)
    x_pad = np.zeros((D, B, ROW), dtype=ml_dtypes.bfloat16)
    x_pad[:, :, PAD:] = x_t
    w_flat = np.ascontiguousarray(w[:, 0, :])  # (D, K) f32

    in_maps = []
    for i in range(N_CORES):
        lo, hi = i * D_LOCAL, (i + 1) * D_LOCAL
        m = {"x": np.ascontiguousarray(x_pad[lo:hi].reshape(G, 128, B, ROW))}
        m["w"] = np.ascontiguousarray(
            w_flat[lo:hi].reshape(G, 128, K).transpose(1, 0, 2).reshape(128, G * K)
        )
        # diag stationaries, laid out [128, G*K*128] partition-first
        wd = np.zeros((G, K, 128, 128), dtype=ml_dtypes.bfloat16)
        wl = w_flat[lo:hi].reshape(G, 128, K).astype(ml_dtypes.bfloat16)
        idx = np.arange(128)
        for g in range(G):
            for k in range(K):
                wd[g, k, idx, idx] = wl[g, :, k]
        # (G,K,p,m) -> (p, G,K,m) -> [128, G*K*128]
        m["wd"] = np.ascontiguousarray(
            wd.transpose(2, 0, 1, 3).reshape(128, G * K * 128)
        )
        in_maps.append(m)
    return in_maps


def _assemble(results):
    parts = []
    for r in results:
        y = np.asarray(r["out"]).reshape(D_LOCAL, B, S)
        parts.append(y)
    y_full = np.concatenate(parts, axis=0)  # (D, B, S) bf16
    return np.ascontiguousarray(y_full.transpose(1, 2, 0)).astype(np.float32)


def kernel(x, w):
    from concourse.bass_utils import run_bass_kernel_spmd

    nc = _get_nc()
    in_maps = _make_in_maps(x, w)
    trace = bool(int(os.environ.get("KERNEL_TRACE", "0")))
    res = None
    err = None
    for attempt in range(3):
        try:
            res = run_bass_kernel_spmd(
                nc, in_maps, core_ids=list(range(N_CORES)),
                trace=trace and attempt == 0,
            )
            break
        except Exception as e:  # transient NRT device errors / missing trace hook
            err = e
            os.environ["BASS_NEVER_TRACE"] = "1"
            trace = False
    if res is None:
        raise err
    _CACHE["last_results"] = res
    return _assemble(res.results)
